# revision 21
# baseline (speedup 1.0000x reference)
"""Trainium2 Bass kernel for nn_CalibrationNetwork (MoE routing over 12 judges).

Strategy: shared + judge-specific weights are pre-summed on the host into 12
effective per-judge MLPs.  Samples are sorted by judge id on the host, each
judge's slots padded to a fixed capacity 2*Cc, and the resulting 24 fixed-size
chunks (2 per judge) are dealt 3-per-core to the 8 NeuronCores.  MLP matmul
operands are bf16 (f32 PSUM accumulation; HW warm rate is 1 cyc/row for both
f32r and bf16, so bf16's win is halved DMA/SBUF traffic, not PE speed).

Layout tricks:
- The per-question softmax runs head-major with n-tile pairs stacked at
  64-row offsets inside one PSUM bank: the av stationary ships as 128-wide
  zero-padded blocks (values in cols 0-63 or 64-127) so every head matmul
  writes the full 128 partitions at base 0 and cross-writes add 0.  Per
  64g-row group: one exp, one block-ones denominator matmul (col 64a+r<35 =
  q-group sum, other cols = tile total so the reciprocal stays finite), one
  reciprocal_approx_fast and one multiply.  The softmax path stays f32/f32r
  for precision at identical warm PE speed.
- Relu evacuations alternate ACT/DVE per n-tile (~balanced against PE);
  ACT issues no DMA at all; inputs ride the SP HWDGE queue (weight blobs,
  bias) and the SWDGE queue (x, ones); one output DMA per softmax group.
- The PE warms up on a Pool-memset dummy tile during the initial DMA wait
  (NWARM sized so warmup ends right as chunk-0 data lands), and emission is
  software-pipelined (softmax of chunk s emitted after l1 of chunk s+1) so
  the PE never idles long enough for the HAM clock gate to re-throttle
  mid-kernel -- HAM oscillation was the dominant loss in the f32r baseline.

ldw-opt is NOT enabled: this neuronxcc build rejects bf16 LDWEIGHTS under
--enable-ldw-opt=true; per-matmul LDWEIGHTS are fully hidden by the PE's
weight-load pull-ahead anyway (measured warm spacing 215ns = N/2.4GHz).
"""

import os
import sys

import numpy as np

for _p in ("/opt/trn_rl_repo", "/root/.axon_site/_ro/trn_rl_repo"):
    if os.path.isdir(_p) and _p not in sys.path:
        sys.path.insert(0, _p)

B, D, H1, H2, J, Q, O = 32768, 35, 256, 256, 12, 7, 5
NCORES = 8
SEG = 3                    # chunks per core
NCHUNKS = NCORES * SEG     # 24 = 2 chunks per judge
QO = Q * O                 # 35
QOp = QO + 1               # padded head dim
NWB = 512 + 512            # weight-blob cols per chunk: a2 | av-blocks

NWARM = 5                  # PE warmup matmuls (HAM ramp during DMA wait)
TRACE = False              # set True in test harness to collect NTFF profile
LAST_RESULTS = None        # BassKernelResults of the last run (for test.py)

_PROG_CACHE = {}


def _ngroups(NT):
    """n-tile groups: pairs stacked at 64-row offsets in one PSUM bank."""
    gs = []
    n = 0
    while n < NT:
        g = min(2, NT - n)
        gs.append((n, g))
        n += g
    return gs


def _build_program(Cc):
    import concourse.bass as bass
    import concourse.tile as tile
    from concourse import bacc, mybir

    f32 = mybir.dt.float32
    f32r = mybir.dt.float32r
    bf16 = mybir.dt.bfloat16
    AF = mybir.ActivationFunctionType
    ALU = mybir.AluOpType

    NT = Cc // 512            # 512-wide n-tiles per chunk
    groups = _ngroups(NT)
    NGR = len(groups)

    nc = bacc.Bacc(None, target_bir_lowering=False, debug=False, num_swdge_queues=1)

    a1_d = nc.dram_tensor("a1", [D + 1, SEG, 256], bf16, kind="ExternalInput")
    xt_d = nc.dram_tensor("xt", [D + 1, SEG * Cc], bf16, kind="ExternalInput")
    wb_d = nc.dram_tensor("wb", [128, SEG, NWB], bf16, kind="ExternalInput")
    bias_d = nc.dram_tensor("bias", [128, SEG, 3], f32, kind="ExternalInput")
    ones_d = nc.dram_tensor("ones", [128, 128], f32r, kind="ExternalInput")
    out_d = nc.dram_tensor("out", [64 * NT, SEG, 512], bf16, kind="ExternalOutput")

    with nc.allow_low_precision(reason="bf16/f32r matmul operands are intentional"), \
            tile.TileContext(nc) as tc:
        with (
            tc.tile_pool(name="cst", bufs=1) as cst,
            tc.tile_pool(name="wp", bufs=1) as wp,
            tc.tile_pool(name="zp", bufs=2) as zp,
            tc.tile_pool(name="op", bufs=2) as op_,
            tc.tile_pool(name="ps", bufs=8, space="PSUM") as ps,
        ):
            # PE warmup: zero-matmuls keep the HAM activity window busy
            # during the initial DMA wait so real matmuls start ramped
            wmv = cst.tile([128, 512], bf16, name="wmv")
            nc.gpsimd.memset(wmv[:], 0.0)
            for w in range(NWARM):
                wps = ps.tile([128, 512], f32, tag="big", name=f"warm_{w}")
                nc.tensor.matmul(wps[:], wmv[:, :128], wmv[:], start=True, stop=True)

            # input loads, ALL on the SP HWDGE queue (the SWDGE queue stays
            # unused so its drain/teardown work disappears), ordered by when
            # each tensor is first needed: a1 (small) + chunk-0 x first so
            # layer 1 starts ~1.5us earlier than with a1 inside the blob
            xts, wbs = [], []
            a1t = wp.tile([D + 1, SEG, 256], bf16, name="a1t")
            nc.sync.dma_start(a1t[:], a1_d[:])
            for s in range(SEG):
                xts.append(wp.tile([D + 1, Cc], bf16, tag=f"xc{s}", name=f"xc_{s}"))
                wbs.append(wp.tile([128, NWB], bf16, tag=f"wb{s}", name=f"wb_{s}"))
            bt = cst.tile([128, SEG, 3], f32, name="bt")
            ones_g = cst.tile([128, 128], f32r, name="ones_g")
            # chunk-0 x ships per n-tile so l1 starts on the first 36KB;
            # ones rides early (den(0) needs it ~13us -- landing last risked
            # an in-order PE stall mid-HAM-ramp)
            for n in range(NT):
                nc.sync.dma_start(
                    xts[0][:, n * 512 : (n + 1) * 512],
                    xt_d[:, n * 512 : (n + 1) * 512],
                )
            nc.sync.dma_start(wbs[0][:], wb_d[:, 0])
            nc.sync.dma_start(bt[:], bias_d[:])
            nc.sync.dma_start(ones_g[:], ones_d[:])
            nc.sync.dma_start(wbs[1][:], wb_d[:, 1])
            nc.sync.dma_start(xts[1][:], xt_d[:, Cc : 2 * Cc])
            nc.sync.dma_start(wbs[2][:], wb_d[:, 2])
            nc.sync.dma_start(xts[2][:], xt_d[:, 2 * Cc : 3 * Cc])

            z1s = [None] * SEG
            z2s = [None] * SEG
            e3s = [dict() for _ in range(SEG)]

            def a2_ap(s, k, m):
                return wbs[s][:, k * 256 + m * 128 : k * 256 + (m + 1) * 128]

            def a1_ap(s, m):
                return a1t[:, s, m * 128 : (m + 1) * 128]

            def av_ap(s, k, blk):
                c = 512 + (2 * k + blk) * 128
                return wbs[s][:, c : c + 128]

            def relu_evac(dst, src, idx, bias=None):
                if idx % 2 == 0:
                    nc.scalar.activation(
                        dst, src, AF.Relu,
                        **({"bias": bias} if bias is not None else {}),
                    )
                elif bias is None:
                    nc.vector.tensor_scalar(
                        out=dst, in0=src, scalar1=0.0, scalar2=None, op0=ALU.max,
                    )
                else:
                    nc.vector.tensor_scalar(
                        out=dst, in0=src, scalar1=bias, scalar2=0.0,
                        op0=ALU.add, op1=ALU.max,
                    )

            def emit_l1(s):
                xt = xts[s]
                z1 = zp.tile([128, 2, Cc], bf16, tag="z1", name=f"z1_{s}")
                z1s[s] = z1
                for m in range(2):
                    for n in range(NT):
                        p1 = ps.tile([128, 512], f32, tag="big",
                                     name=f"p1_{s}_{m}_{n}")
                        nc.tensor.matmul(
                            p1[:],
                            a1_ap(s, m),
                            xt[:, n * 512 : (n + 1) * 512],
                            start=True,
                            stop=True,
                        )
                        relu_evac(z1[:, m, n * 512 : (n + 1) * 512], p1[:],
                                  m * NT + n)

            def emit_l2(s):
                z1 = z1s[s]
                z2 = zp.tile([128, 2, Cc], bf16, tag="z2", name=f"z2_{s}")
                z2s[s] = z2
                for m in range(2):
                    p2s = {}
                    for k in range(2):
                        for n in range(NT):
                            if k == 0:
                                p2s[n] = ps.tile(
                                    [128, 512], f32, tag="big",
                                    name=f"p2_{s}_{m}_{n}",
                                )
                            nc.tensor.matmul(
                                p2s[n][:],
                                a2_ap(s, k, m),
                                z1[:, k, n * 512 : (n + 1) * 512],
                                start=(k == 0),
                                stop=(k == 1),
                            )
                            if k == 1:
                                relu_evac(
                                    z2[:, m, n * 512 : (n + 1) * 512],
                                    p2s[n][:], m * NT + n + 1,
                                    bias=bt[:, s, m : m + 1],
                                )

            def emit_hd(s):
                # heads: pairs of n-tiles stacked at 64-row offsets in one
                # PSUM bank.  The av stationary comes in two 128-wide blocks
                # (values in cols 0-63 or 64-127, zeros elsewhere) so every
                # matmul writes the full 128 partitions at base 0 -- tile a's
                # rows see +0 from tile b's matmuls and vice versa.
                z2 = z2s[s]
                for n0, g in groups:
                    ph = ps.tile([128, 512], f32, tag="big", name=f"ph_{s}_{n0}")
                    for a in range(g):
                        n = n0 + a
                        for k in range(2):
                            nc.tensor.matmul(
                                ph[:],
                                av_ap(s, k, a),
                                z2[:, k, n * 512 : (n + 1) * 512],
                                start=(a == 0 and k == 0),
                                stop=(a == g - 1 and k == 1),
                            )
                    e3 = op_.tile([64 * g, 512], f32r, tag="e", bufs=4,
                                  name=f"e_{s}_{n0}")
                    e3s[s][n0] = e3
                    nc.scalar.activation(
                        e3[:], ph[0 : 64 * g, :], AF.Exp,
                        bias=bt[0 : 64 * g, s, 2:3],
                    )

            def emit_sm(s):
                # one block-ones matmul per group: row 64a+r (r<35) = group-q
                # denominator, rows 64a+35.. = tile total (keeps recip finite)
                for gi, (n0, g) in enumerate(groups):
                    e3 = e3s[s][n0]
                    dn = ps.tile([64 * g, 512], f32, tag="big", name=f"dn_{s}_{n0}")
                    nc.tensor.matmul(
                        dn[:], ones_g[0 : 64 * g, 0 : 64 * g], e3[:],
                        start=True, stop=True,
                    )
                    rec = op_.tile([64 * g, 512], f32, tag="rec", name=f"rec_{s}_{n0}")
                    nc.vector.reciprocal_approx_fast(rec[:], dn[:])
                    outm = op_.tile([64 * g, 512], bf16, tag="outm",
                                    name=f"outm_{s}_{n0}")
                    nc.vector.tensor_tensor(outm[:], e3[:], rec[:], ALU.mult)
                    nc.sync.dma_start(
                        out_d[128 * gi : 128 * gi + 64 * g, s, :], outm[:]
                    )

            # software-pipelined emission: softmax of chunk s is emitted after
            # l1 of chunk s+1 so the PE has work while ACT runs exp(s)
            emit_l1(0)
            emit_l2(0)
            emit_hd(0)
            for s in range(SEG):
                if s + 1 < SEG:
                    emit_l1(s + 1)
                emit_sm(s)
                if s + 1 < SEG:
                    emit_l2(s + 1)
                    emit_hd(s + 1)

    nc.compile()
    return nc


def _get_program(Cc):
    if Cc not in _PROG_CACHE:
        _PROG_CACHE[Cc] = _build_program(Cc)
    return _PROG_CACHE[Cc]


def kernel(**inputs):
    global LAST_RESULTS
    import ml_dtypes

    bf16 = ml_dtypes.bfloat16

    x = np.ascontiguousarray(np.asarray(inputs["x"], dtype=np.float32))
    ids = np.asarray(inputs["judge_ids"]).astype(np.int64).ravel()
    W1_w = np.asarray(inputs["W1_w"], np.float32)
    W1_b = np.asarray(inputs["W1_b"], np.float32)
    W2_w = np.asarray(inputs["W2_w"], np.float32)
    W2_b = np.asarray(inputs["W2_b"], np.float32)
    W1a_w = np.asarray(inputs["W1a_w"], np.float32)
    W1a_b = np.asarray(inputs["W1a_b"], np.float32)
    W2a_w = np.asarray(inputs["W2a_w"], np.float32)
    W2a_b = np.asarray(inputs["W2a_b"], np.float32)
    V_w = np.asarray(inputs["V_w"], np.float32)
    V_b = np.asarray(inputs["V_b"], np.float32)
    Va_w = np.asarray(inputs["Va_w"], np.float32)
    Va_b = np.asarray(inputs["Va_b"], np.float32)

    Bx = x.shape[0]
    cnts = np.bincount(ids, minlength=J)
    Cc = 1536
    mx = int(cnts.max())
    if 2 * Cc < mx:
        Cc = ((mx + 1) // 2 + 511) // 512 * 512
    NT = Cc // 512
    groups = _ngroups(NT)
    NGR = len(groups)

    # effective per-judge weights (shared + judge-specific, biases folded)
    A1 = (W1_w[None] + W1a_w).copy()                      # (J, H1, D+1)
    A1[:, :, D] += W1_b[None] + W1a_b
    A2 = W2_w[None] + W2a_w                               # (J, H2, H1+1)
    b2 = A2[:, :, H1] + W2_b[None] + W2a_b                # (J, H2)
    A2c = A2[:, :, :H1]                                   # (J, H2, H1)
    AV = (V_w[None] + Va_w).reshape(J, QO, H2 + 1)
    bV = (AV[:, :, H2] + (V_b[None] + Va_b).reshape(J, QO)).astype(np.float32)
    AVc = AV[:, :, :H2]

    # SBUF layouts (bf16)
    a1sb = np.transpose(A1, (0, 2, 1)).astype(bf16)       # (J, 36, 256)
    a2sb = np.transpose(
        A2c.reshape(J, H2, 2, 128), (0, 3, 2, 1)
    ).astype(bf16)                                        # (J,128,2,256)
    b2sb = np.ascontiguousarray(np.transpose(b2.reshape(J, 2, 128), (0, 2, 1)))
    avsb = np.transpose(AVc.reshape(J, QO, 2, 128), (0, 3, 2, 1))  # (J,128,2,35)

    # weight blob [128, NWB]: a2 (512 cols) | av blocks (4 x 128 cols,
    # values in cols 0-63 or 64-127 of each block, zeros elsewhere)
    wblob = np.zeros((J, 128, NWB), bf16)
    wblob[:, :, :512] = a2sb.reshape(J, 128, 512)
    avblk = np.zeros((J, 128, 2, 2, 128), np.float32)
    avblk[:, :, :, 0, :QO] = avsb
    avblk[:, :, :, 1, 64 : 64 + QO] = avsb
    wblob[:, :, 512:] = avblk.reshape(J, 128, 512)

    # bias blob [128, 3]: b2 m0 | b2 m1 | 64-stride-replicated head bias
    bblob = np.zeros((J, 128, 3), np.float32)
    bblob[:, :, :2] = b2sb
    bvp64 = np.full((J, 64), -1e30, np.float32)
    bvp64[:, :QO] = bV
    bblob[:, :, 2] = np.tile(bvp64, (1, 2))

    # block-ones matrix: ones_g[64a+k, 64a+r] = 1 iff same q-group
    # (k,r < 35); cols r >= 35 get the tile total (keeps recip finite)
    ones_g = np.zeros((128, 128), np.float32)
    for a in range(2):
        for r in range(64):
            if r < QO:
                q = r // O
                ones_g[64 * a + q * O : 64 * a + (q + 1) * O, 64 * a + r] = 1.0
            else:
                ones_g[64 * a : 64 * a + QO, 64 * a + r] = 1.0

    # slot -> sample map: judge j owns slots [j*2Cc, (j+1)*2Cc)
    order = np.argsort(ids, kind="stable")
    slot2samp = np.full(NCHUNKS * Cc, -1, np.int64)
    pos = 0
    for j in range(J):
        k = int(cnts[j])
        slot2samp[j * 2 * Cc : j * 2 * Cc + k] = order[pos : pos + k]
        pos += k
    chunk_judge = np.repeat(np.arange(J), 2)

    in_maps = []
    core_meta = []
    for c in range(NCORES):
        sl = slot2samp[c * SEG * Cc : (c + 1) * SEG * Cc]
        valid = sl >= 0
        Xc = np.zeros((SEG * Cc, D + 1), np.float32)
        Xc[valid, :D] = x[sl[valid]]
        Xc[:, D] = 1.0
        js = chunk_judge[c * SEG : (c + 1) * SEG]
        in_maps.append(
            {
                "a1": np.ascontiguousarray(np.transpose(a1sb[js], (1, 0, 2))),
                "xt": np.ascontiguousarray(Xc.T.astype(bf16)),
                "wb": np.ascontiguousarray(np.transpose(wblob[js], (1, 0, 2))),
                "bias": np.ascontiguousarray(np.transpose(bblob[js], (1, 0, 2))),
                "ones": ones_g,
            }
        )
        core_meta.append((sl, valid))

    # ldw-opt stays disabled: this neuronxcc build rejects bf16 LDWEIGHTS
    # under --enable-ldw-opt=true
    nc = _get_program(Cc)
    from concourse.bass_utils import run_bass_kernel_spmd

    res = run_bass_kernel_spmd(
        nc,
        in_maps,
        core_ids=list(range(NCORES)),
        trace=TRACE,
    )
    LAST_RESULTS = res

    full = np.zeros((Bx, Q, O), np.float32)
    for c in range(NCORES):
        # out [64*NT, SEG, 512]: row 64n'+r (r<35) of chunk s col c is the
        # sample (s*Cc + n*512 + c) head row r, n' the 64-stride slot of n
        arr = np.asarray(res.results[c]["out"]).astype(np.float32)
        oc = arr.reshape(NT, 64, SEG, 512)[:, :QO]              # (n, r, s, c)
        oc = oc.transpose(2, 0, 3, 1).reshape(SEG * Cc, QO)     # (s,n,c), r
        sl, valid = core_meta[c]
        full[sl[valid]] = oc[valid].reshape(-1, Q, O)
    return full


# revision 23
# speedup vs baseline: 1.1760x; 1.1760x over previous
"""Trainium2 Bass kernel for nn_CalibrationNetwork (MoE routing over 12 judges).

Strategy: shared + judge-specific weights are pre-summed on the host into 12
effective per-judge MLPs.  Samples are sorted by judge id on the host, each
judge's slots padded to a fixed capacity 2*Cc, and the resulting 24 fixed-size
chunks (2 per judge) are dealt 3-per-core to the 8 NeuronCores.  MLP matmul
operands are bf16 (f32 PSUM accumulation; HW warm rate is 1 cyc/row for both
f32r and bf16, so bf16's win is halved DMA/SBUF traffic, not PE speed).

Layout tricks:
- The per-question softmax runs head-major with n-tile pairs stacked at
  64-row offsets inside one PSUM bank: the av stationary ships as 128-wide
  zero-padded blocks (values in cols 0-63 or 64-127) so every head matmul
  writes the full 128 partitions at base 0 and cross-writes add 0.  Per
  64g-row group: one exp, one block-ones denominator matmul (col 64a+r<35 =
  q-group sum, other cols = tile total so the reciprocal stays finite), one
  reciprocal_approx_fast and one multiply.  The softmax path stays f32/f32r
  for precision at identical warm PE speed.
- Relu evacuations alternate ACT/DVE per n-tile (~balanced against PE);
  ACT issues no DMA at all; inputs ride the SP HWDGE queue (weight blobs,
  bias) and the SWDGE queue (x, ones); one output DMA per softmax group.
- The PE warms up on a Pool-memset dummy tile during the initial DMA wait
  (NWARM sized so warmup ends right as chunk-0 data lands), and emission is
  software-pipelined (softmax of chunk s emitted after l1 of chunk s+1) so
  the PE never idles long enough for the HAM clock gate to re-throttle
  mid-kernel -- HAM oscillation was the dominant loss in the f32r baseline.

ldw-opt is NOT enabled: this neuronxcc build rejects bf16 LDWEIGHTS under
--enable-ldw-opt=true; per-matmul LDWEIGHTS are fully hidden by the PE's
weight-load pull-ahead anyway (measured warm spacing 215ns = N/2.4GHz).
"""

import os
import sys

import numpy as np

for _p in ("/opt/trn_rl_repo", "/root/.axon_site/_ro/trn_rl_repo"):
    if os.path.isdir(_p) and _p not in sys.path:
        sys.path.insert(0, _p)

B, D, H1, H2, J, Q, O = 32768, 35, 256, 256, 12, 7, 5
NCORES = 8
SEG = 3                    # chunks per core
NCHUNKS = NCORES * SEG     # 24 = 2 chunks per judge
QO = Q * O                 # 35
QOp = QO + 1               # padded head dim
NWB = 512 + 512            # weight-blob cols per chunk: a2 | av-blocks

NWARM = 5                  # PE warmup matmuls (HAM ramp during DMA wait)
TRACE = False              # set True in test harness to collect NTFF profile
LAST_RESULTS = None        # BassKernelResults of the last run (for test.py)

_PROG_CACHE = {}


def _ngroups(NT):
    """n-tile groups: pairs stacked at 64-row offsets in one PSUM bank."""
    gs = []
    n = 0
    while n < NT:
        g = min(2, NT - n)
        gs.append((n, g))
        n += g
    return gs


def _build_program(Cc):
    import concourse.bass as bass
    import concourse.tile as tile
    from concourse import bacc, mybir

    f32 = mybir.dt.float32
    f32r = mybir.dt.float32r
    bf16 = mybir.dt.bfloat16
    AF = mybir.ActivationFunctionType
    ALU = mybir.AluOpType

    NT = Cc // 512            # 512-wide n-tiles per chunk
    groups = _ngroups(NT)
    NGR = len(groups)

    nc = bacc.Bacc(None, target_bir_lowering=False, debug=False, num_swdge_queues=1)

    a1_d = nc.dram_tensor("a1", [D + 1, SEG, 256], bf16, kind="ExternalInput")
    xt_d = nc.dram_tensor("xt", [D + 1, SEG * Cc], bf16, kind="ExternalInput")
    wb_d = nc.dram_tensor("wb", [128, SEG, NWB], bf16, kind="ExternalInput")
    bias_d = nc.dram_tensor("bias", [128, SEG, 3], f32, kind="ExternalInput")
    ones_d = nc.dram_tensor("ones", [128, 128], f32r, kind="ExternalInput")
    out_d = nc.dram_tensor("out", [64 * NT, SEG, 512], bf16, kind="ExternalOutput")

    with nc.allow_low_precision(reason="bf16/f32r matmul operands are intentional"), \
            tile.TileContext(nc) as tc:
        with (
            tc.tile_pool(name="cst", bufs=1) as cst,
            tc.tile_pool(name="wp", bufs=1) as wp,
            tc.tile_pool(name="zp", bufs=2) as zp,
            tc.tile_pool(name="op", bufs=2) as op_,
            tc.tile_pool(name="ps", bufs=8, space="PSUM") as ps,
        ):
            # PE warmup: zero-matmuls keep the HAM activity window busy
            # during the initial DMA wait so real matmuls start ramped
            wmv = cst.tile([128, 512], bf16, name="wmv")
            nc.gpsimd.memset(wmv[:], 0.0)
            for w in range(NWARM):
                wps = ps.tile([128, 512], f32, tag="big", name=f"warm_{w}")
                nc.tensor.matmul(wps[:], wmv[:, :128], wmv[:], start=True, stop=True)

            # input loads, ALL on the SP HWDGE queue (the SWDGE queue stays
            # unused so its drain/teardown work disappears), ordered by when
            # each tensor is first needed: a1 (small) + chunk-0 x first so
            # layer 1 starts ~1.5us earlier than with a1 inside the blob
            xts, wbs = [], []
            a1t = wp.tile([D + 1, SEG, 256], bf16, name="a1t")
            nc.sync.dma_start(a1t[:], a1_d[:])
            for s in range(SEG):
                xts.append(wp.tile([D + 1, Cc], bf16, tag=f"xc{s}", name=f"xc_{s}"))
                wbs.append(wp.tile([128, NWB], bf16, tag=f"wb{s}", name=f"wb_{s}"))
            bt = cst.tile([128, SEG, 3], f32, name="bt")
            ones_g = cst.tile([128, 128], f32r, name="ones_g")
            # chunk-0 x ships per n-tile so l1 starts on the first 36KB;
            # ones rides early (den(0) needs it ~13us -- landing last risked
            # an in-order PE stall mid-HAM-ramp)
            for n in range(NT):
                nc.sync.dma_start(
                    xts[0][:, n * 512 : (n + 1) * 512],
                    xt_d[:, n * 512 : (n + 1) * 512],
                )
            nc.sync.dma_start(wbs[0][:], wb_d[:, 0])
            nc.sync.dma_start(bt[:], bias_d[:])
            nc.sync.dma_start(ones_g[:], ones_d[:])
            nc.sync.dma_start(wbs[1][:], wb_d[:, 1])
            nc.sync.dma_start(xts[1][:], xt_d[:, Cc : 2 * Cc])
            nc.sync.dma_start(wbs[2][:], wb_d[:, 2])
            nc.sync.dma_start(xts[2][:], xt_d[:, 2 * Cc : 3 * Cc])

            z1s = [None] * SEG
            z2s = [None] * SEG
            e3s = [dict() for _ in range(SEG)]

            def a2_ap(s, k, m):
                return wbs[s][:, k * 256 + m * 128 : k * 256 + (m + 1) * 128]

            def a1_ap(s, m):
                return a1t[:, s, m * 128 : (m + 1) * 128]

            def av_ap(s, k, blk):
                c = 512 + (2 * k + blk) * 128
                return wbs[s][:, c : c + 128]

            def relu_evac(dst, src, idx, bias=None):
                if idx % 2 == 0:
                    nc.scalar.activation(
                        dst, src, AF.Relu,
                        **({"bias": bias} if bias is not None else {}),
                    )
                elif bias is None:
                    nc.vector.tensor_scalar(
                        out=dst, in0=src, scalar1=0.0, scalar2=None, op0=ALU.max,
                    )
                else:
                    nc.vector.tensor_scalar(
                        out=dst, in0=src, scalar1=bias, scalar2=0.0,
                        op0=ALU.add, op1=ALU.max,
                    )

            def emit_l1(s):
                xt = xts[s]
                z1 = zp.tile([128, 2, Cc], bf16, tag="z1", name=f"z1_{s}")
                z1s[s] = z1
                for m in range(2):
                    for n in range(NT):
                        p1 = ps.tile([128, 512], f32, tag="big",
                                     name=f"p1_{s}_{m}_{n}")
                        nc.tensor.matmul(
                            p1[:],
                            a1_ap(s, m),
                            xt[:, n * 512 : (n + 1) * 512],
                            start=True,
                            stop=True,
                        )
                        relu_evac(z1[:, m, n * 512 : (n + 1) * 512], p1[:],
                                  m * NT + n)

            def emit_l2(s):
                z1 = z1s[s]
                z2 = zp.tile([128, 2, Cc], bf16, tag="z2", name=f"z2_{s}")
                z2s[s] = z2
                for m in range(2):
                    p2s = {}
                    for k in range(2):
                        for n in range(NT):
                            if k == 0:
                                p2s[n] = ps.tile(
                                    [128, 512], f32, tag="big",
                                    name=f"p2_{s}_{m}_{n}",
                                )
                            nc.tensor.matmul(
                                p2s[n][:],
                                a2_ap(s, k, m),
                                z1[:, k, n * 512 : (n + 1) * 512],
                                start=(k == 0),
                                stop=(k == 1),
                            )
                            if k == 1:
                                relu_evac(
                                    z2[:, m, n * 512 : (n + 1) * 512],
                                    p2s[n][:], m * NT + n + 1,
                                    bias=bt[:, s, m : m + 1],
                                )

            def emit_hd(s):
                # heads: pairs of n-tiles stacked at 64-row offsets in one
                # PSUM bank.  The av stationary comes in two 128-wide blocks
                # (values in cols 0-63 or 64-127, zeros elsewhere) so every
                # matmul writes the full 128 partitions at base 0 -- tile a's
                # rows see +0 from tile b's matmuls and vice versa.
                z2 = z2s[s]
                for n0, g in groups:
                    ph = ps.tile([128, 512], f32, tag="big", name=f"ph_{s}_{n0}")
                    for a in range(g):
                        n = n0 + a
                        for k in range(2):
                            nc.tensor.matmul(
                                ph[:],
                                av_ap(s, k, a),
                                z2[:, k, n * 512 : (n + 1) * 512],
                                start=(a == 0 and k == 0),
                                stop=(a == g - 1 and k == 1),
                            )
                    e3 = op_.tile([64 * g, 512], f32r, tag="e", bufs=4,
                                  name=f"e_{s}_{n0}")
                    e3s[s][n0] = e3
                    nc.scalar.activation(
                        e3[:], ph[0 : 64 * g, :], AF.Exp,
                        bias=bt[0 : 64 * g, s, 2:3],
                    )

            def emit_sm(s):
                # one block-ones matmul per group: row 64a+r (r<35) = group-q
                # denominator, rows 64a+35.. = tile total (keeps recip finite)
                for gi, (n0, g) in enumerate(groups):
                    e3 = e3s[s][n0]
                    dn = ps.tile([64 * g, 512], f32, tag="big", name=f"dn_{s}_{n0}")
                    nc.tensor.matmul(
                        dn[:], ones_g[0 : 64 * g, 0 : 64 * g], e3[:],
                        start=True, stop=True,
                    )
                    rec = op_.tile([64 * g, 512], f32, tag="rec", name=f"rec_{s}_{n0}")
                    nc.vector.reciprocal_approx_fast(rec[:], dn[:])
                    outm = op_.tile([64 * g, 512], bf16, tag="outm",
                                    name=f"outm_{s}_{n0}")
                    nc.vector.tensor_tensor(outm[:], e3[:], rec[:], ALU.mult)
                    nc.sync.dma_start(
                        out_d[128 * gi : 128 * gi + 64 * g, s, :], outm[:]
                    )

            # software-pipelined emission: softmax of chunk s is emitted after
            # l1 of chunk s+1 so the PE has work while ACT runs exp(s)
            emit_l1(0)
            emit_l2(0)
            emit_hd(0)
            for s in range(SEG):
                if s + 1 < SEG:
                    emit_l1(s + 1)
                emit_sm(s)
                if s + 1 < SEG:
                    emit_l2(s + 1)
                    emit_hd(s + 1)

    nc.compile()
    return nc


def _get_program(Cc):
    if Cc not in _PROG_CACHE:
        _PROG_CACHE[Cc] = _build_program(Cc)
    return _PROG_CACHE[Cc]


def kernel(**inputs):
    global LAST_RESULTS
    import ml_dtypes

    bf16 = ml_dtypes.bfloat16

    x = np.ascontiguousarray(np.asarray(inputs["x"], dtype=np.float32))
    ids = np.asarray(inputs["judge_ids"]).astype(np.int64).ravel()
    W1_w = np.asarray(inputs["W1_w"], np.float32)
    W1_b = np.asarray(inputs["W1_b"], np.float32)
    W2_w = np.asarray(inputs["W2_w"], np.float32)
    W2_b = np.asarray(inputs["W2_b"], np.float32)
    W1a_w = np.asarray(inputs["W1a_w"], np.float32)
    W1a_b = np.asarray(inputs["W1a_b"], np.float32)
    W2a_w = np.asarray(inputs["W2a_w"], np.float32)
    W2a_b = np.asarray(inputs["W2a_b"], np.float32)
    V_w = np.asarray(inputs["V_w"], np.float32)
    V_b = np.asarray(inputs["V_b"], np.float32)
    Va_w = np.asarray(inputs["Va_w"], np.float32)
    Va_b = np.asarray(inputs["Va_b"], np.float32)

    Bx = x.shape[0]
    cnts = np.bincount(ids, minlength=J)
    Cc = 1536
    mx = int(cnts.max())
    if 2 * Cc < mx:
        Cc = ((mx + 1) // 2 + 511) // 512 * 512
    NT = Cc // 512
    groups = _ngroups(NT)
    NGR = len(groups)

    # effective per-judge weights (shared + judge-specific, biases folded)
    A1 = (W1_w[None] + W1a_w).copy()                      # (J, H1, D+1)
    A1[:, :, D] += W1_b[None] + W1a_b
    A2 = W2_w[None] + W2a_w                               # (J, H2, H1+1)
    b2 = A2[:, :, H1] + W2_b[None] + W2a_b                # (J, H2)
    A2c = A2[:, :, :H1]                                   # (J, H2, H1)
    AV = (V_w[None] + Va_w).reshape(J, QO, H2 + 1)
    bV = (AV[:, :, H2] + (V_b[None] + Va_b).reshape(J, QO)).astype(np.float32)
    AVc = AV[:, :, :H2]

    # SBUF layouts (bf16)
    a1sb = np.transpose(A1, (0, 2, 1)).astype(bf16)       # (J, 36, 256)
    a2sb = np.transpose(
        A2c.reshape(J, H2, 2, 128), (0, 3, 2, 1)
    ).astype(bf16)                                        # (J,128,2,256)
    b2sb = np.ascontiguousarray(np.transpose(b2.reshape(J, 2, 128), (0, 2, 1)))
    avsb = np.transpose(AVc.reshape(J, QO, 2, 128), (0, 3, 2, 1))  # (J,128,2,35)

    # weight blob [128, NWB]: a2 (512 cols) | av blocks (4 x 128 cols,
    # values in cols 0-63 or 64-127 of each block, zeros elsewhere)
    wblob = np.zeros((J, 128, NWB), bf16)
    wblob[:, :, :512] = a2sb.reshape(J, 128, 512)
    avblk = np.zeros((J, 128, 2, 2, 128), np.float32)
    avblk[:, :, :, 0, :QO] = avsb
    avblk[:, :, :, 1, 64 : 64 + QO] = avsb
    wblob[:, :, 512:] = avblk.reshape(J, 128, 512)

    # bias blob [128, 3]: b2 m0 | b2 m1 | 64-stride-replicated head bias
    bblob = np.zeros((J, 128, 3), np.float32)
    bblob[:, :, :2] = b2sb
    bvp64 = np.full((J, 64), -1e30, np.float32)
    bvp64[:, :QO] = bV
    bblob[:, :, 2] = np.tile(bvp64, (1, 2))

    # block-ones matrix: ones_g[64a+k, 64a+r] = 1 iff same q-group
    # (k,r < 35); cols r >= 35 get the tile total (keeps recip finite)
    ones_g = np.zeros((128, 128), np.float32)
    for a in range(2):
        for r in range(64):
            if r < QO:
                q = r // O
                ones_g[64 * a + q * O : 64 * a + (q + 1) * O, 64 * a + r] = 1.0
            else:
                ones_g[64 * a : 64 * a + QO, 64 * a + r] = 1.0

    # slot -> sample map: judge j owns slots [j*2Cc, (j+1)*2Cc)
    order = np.argsort(ids, kind="stable")
    slot2samp = np.full(NCHUNKS * Cc, -1, np.int64)
    pos = 0
    for j in range(J):
        k = int(cnts[j])
        slot2samp[j * 2 * Cc : j * 2 * Cc + k] = order[pos : pos + k]
        pos += k
    chunk_judge = np.repeat(np.arange(J), 2)

    in_maps = []
    core_meta = []
    for c in range(NCORES):
        sl = slot2samp[c * SEG * Cc : (c + 1) * SEG * Cc]
        valid = sl >= 0
        Xc = np.zeros((SEG * Cc, D + 1), np.float32)
        Xc[valid, :D] = x[sl[valid]]
        Xc[:, D] = 1.0
        js = chunk_judge[c * SEG : (c + 1) * SEG]
        in_maps.append(
            {
                "a1": np.ascontiguousarray(np.transpose(a1sb[js], (1, 0, 2))),
                "xt": np.ascontiguousarray(Xc.T.astype(bf16)),
                "wb": np.ascontiguousarray(np.transpose(wblob[js], (1, 0, 2))),
                "bias": np.ascontiguousarray(np.transpose(bblob[js], (1, 0, 2))),
                "ones": ones_g,
            }
        )
        core_meta.append((sl, valid))

    # ldw-opt stays disabled: this neuronxcc build rejects bf16 LDWEIGHTS
    # under --enable-ldw-opt=true
    nc = _get_program(Cc)
    from concourse.bass_utils import run_bass_kernel_spmd

    res = run_bass_kernel_spmd(
        nc,
        in_maps,
        core_ids=list(range(NCORES)),
        trace=TRACE,
    )
    LAST_RESULTS = res

    full = np.zeros((Bx, Q, O), np.float32)
    for c in range(NCORES):
        # out [64*NT, SEG, 512]: row 64n'+r (r<35) of chunk s col c is the
        # sample (s*Cc + n*512 + c) head row r, n' the 64-stride slot of n
        arr = np.asarray(res.results[c]["out"]).astype(np.float32)
        oc = arr.reshape(NT, 64, SEG, 512)[:, :QO]              # (n, r, s, c)
        oc = oc.transpose(2, 0, 3, 1).reshape(SEG * Cc, QO)     # (s,n,c), r
        sl, valid = core_meta[c]
        full[sl[valid]] = oc[valid].reshape(-1, Q, O)
    return full
